# revision 4
# baseline (speedup 1.0000x reference)
"""Trainium2 Bass kernel for a CAM (channel-attention) module.

Reference computation (per batch b):
    v    = x[b].reshape(C, H*W)                  # C x N
    e    = v @ v.T                               # C x C Gram matrix
    attn = softmax(rowmax(e) - e, axis=-1)       # == exp(rowmin(e)-e) / rowsum
    out  = gamma * (attn @ v) + x[b]

Sharding: data-parallel over batch B=16 across 8 NeuronCores (2 batches/core,
no cross-core communication).

Per core, per batch:
  - DMA x[b] natural tiles v_ct [128c, 4096n] fp32 (exact; feeds the residual)
  - cast to bf16 working copies (matmul operands)
  - PE-transpose bf16 128x128 blocks -> vT_kt [128n, 512c] streamed chunks
  - energy: 4 PSUM banks accumulate e[m-block, :] over 32 n-tiles (bf16 MMs,
    fp32 PSUM accumulation)
  - softmax: rowmin (DVE) + exp with accum_out rowsum (ACT); U unnormalized
  - PE-transpose U -> UT (bf16); raw[c,n] = sum_d U[c,d] v[d,n] (bf16 MMs)
  - eviction fuses normalization+gamma+residual: out = raw*(gamma/Z) + x_fp32
    (so for gamma == 0 the output is bit-exact x)
"""

import numpy as np

P = 128
C = 512
N = 4096
CT = C // P      # 4 c-tiles
NT = N // P      # 32 n-tiles
CH = 512         # matmul free-dim chunk
NCH = N // CH    # 8 n-chunks
B = 16
NCORES = 8
BPC = B // NCORES  # batches per core

_CACHE = {}


def _build_program():
    import concourse.bacc as bacc
    import concourse.mybir as mybir
    import concourse.tile as tile
    from concourse.masks import make_identity

    f32 = mybir.dt.float32
    bf16 = mybir.dt.bfloat16
    Alu = mybir.AluOpType
    Act = mybir.ActivationFunctionType

    nc = bacc.Bacc("TRN2", target_bir_lowering=False, debug=False)
    x_d = nc.dram_tensor("x", [BPC, C, N], f32, kind="ExternalInput").ap()
    g_d = nc.dram_tensor("gamma", [1], f32, kind="ExternalInput").ap()
    o_d = nc.dram_tensor("out", [BPC, C, N], f32, kind="ExternalOutput").ap()

    with tile.TileContext(nc) as tc:
        with (
            tc.tile_pool(name="const", bufs=1) as const_pool,
            tc.tile_pool(name="vp", bufs=2) as v_pool,
            tc.tile_pool(name="vhp", bufs=1) as vh_pool,
            tc.tile_pool(name="vtp", bufs=4) as vt_pool,
            tc.tile_pool(name="up", bufs=1) as u_pool,
            tc.tile_pool(name="stat", bufs=2) as st_pool,
            tc.tile_pool(name="outp", bufs=2) as out_pool,
            tc.tile_pool(name="psmm", bufs=1, space="PSUM") as ps_mm,
            tc.tile_pool(name="pstp", bufs=2, space="PSUM") as ps_tp,
        ):
            ident_h = const_pool.tile([P, P], bf16, tag="identh")
            make_identity(nc, ident_h)
            ident_f = const_pool.tile([P, P], f32, tag="identf")
            make_identity(nc, ident_f)
            gamma_bc = const_pool.tile([P, 1], f32, tag="gamma")
            nc.sync.dma_start(gamma_bc, g_d.to_broadcast((P, 1)))

            for b in range(BPC):
                # ---------- load x[b]: 4 fp32 tiles [128c, 4096n] ----------
                v_sb = []
                for ct in range(CT):
                    v_t = v_pool.tile([P, N], f32, tag=f"v{ct}", name=f"v_{b}_{ct}")
                    nc.sync.dma_start(v_t, x_d[b, ct * P:(ct + 1) * P, :])
                    v_sb.append(v_t)

                # bf16 working copies (split across ACT and DVE)
                v_h = []
                for ct in range(CT):
                    vh_t = vh_pool.tile([P, N], bf16, tag=f"vh{ct}",
                                        name=f"vh_{b}_{ct}")
                    if ct < 2:
                        nc.scalar.activation(vh_t, v_sb[ct], Act.Copy)
                    else:
                        nc.vector.tensor_copy(vh_t, v_sb[ct])
                    v_h.append(vh_t)

                mins = st_pool.tile([P, CT], f32, tag="mins", name=f"mins_{b}")
                zsum = st_pool.tile([P, CT], f32, tag="zsum", name=f"zsum_{b}")
                gz = st_pool.tile([P, CT], f32, tag="gz", name=f"gz_{b}")
                u_sb = u_pool.tile([P, CT, C], f32, tag="u", name=f"u_{b}")
                ut_sb = u_pool.tile([P, CT, C], bf16, tag="ut", name=f"ut_{b}")

                # ---------- energy = v @ v.T, streamed over n ----------
                ps_e = [
                    ps_mm.tile([P, CH], f32, tag=f"mm{m}", name=f"ps_e_{b}_{m}")
                    for m in range(CT)
                ]
                for kt in range(NT):
                    ps_t = ps_tp.tile([P, CH], bf16, tag="tp", name=f"ps_tv_{b}_{kt}")
                    for ct in range(CT):
                        nc.tensor.transpose(
                            ps_t[:, ct * P:(ct + 1) * P],
                            v_h[ct][:, kt * P:(kt + 1) * P],
                            ident_h,
                        )
                    vT_kt = vt_pool.tile([P, C], bf16, tag="vt", name=f"vT_{b}_{kt}")
                    nc.scalar.activation(vT_kt, ps_t, Act.Copy)
                    for m in range(CT):
                        nc.tensor.matmul(
                            ps_e[m],
                            vT_kt[:, m * P:(m + 1) * P],
                            vT_kt[:],
                            start=(kt == 0),
                            stop=(kt == NT - 1),
                        )

                # ---------- softmax (unnormalized U; rowsum via accum) ----------
                for m in range(CT):
                    nc.vector.tensor_reduce(
                        mins[:, m:m + 1], ps_e[m], axis=mybir.AxisListType.X,
                        op=Alu.min,
                    )
                    nc.scalar.activation(
                        u_sb[:, m, :], ps_e[m], Act.Exp,
                        bias=mins[:, m:m + 1], scale=-1.0,
                        accum_out=zsum[:, m:m + 1],
                    )
                    # gz = gamma / rowsum
                    nc.vector.reciprocal(gz[:, m:m + 1], zsum[:, m:m + 1])
                    nc.vector.tensor_tensor(
                        gz[:, m:m + 1], gz[:, m:m + 1], gamma_bc, Alu.mult,
                    )

                # ---------- UT = U.T (16 PE transposes, fp32 in -> bf16 out) ----
                for kt in range(CT):
                    ps_u = ps_tp.tile([P, CH], f32, tag="tpu", name=f"ps_ut_{b}_{kt}")
                    for m in range(CT):
                        nc.tensor.transpose(
                            ps_u[:, m * P:(m + 1) * P],
                            u_sb[:, m, kt * P:(kt + 1) * P],
                            ident_f,
                        )
                    nc.scalar.activation(ut_sb[:, kt, :], ps_u, Act.Copy)

                # ---------- raw[c,n] = sum_d U[c,d] v[d,n]; fused eviction ----
                for m in range(CT):
                    for half in range(2):
                        out_t = out_pool.tile([P, N // 2], f32, tag="o",
                                              name=f"o_{b}_{m}_{half}")
                        ps_o = [
                            ps_mm.tile([P, CH], f32, tag=f"mm{j}",
                                       name=f"ps_o_{b}_{m}_{half}_{j}")
                            for j in range(4)
                        ]
                        for kt in range(CT):
                            for j in range(4):
                                ch = half * 4 + j
                                nc.tensor.matmul(
                                    ps_o[j],
                                    ut_sb[:, kt, m * P:(m + 1) * P],
                                    v_h[kt][:, ch * CH:(ch + 1) * CH],
                                    start=(kt == 0),
                                    stop=(kt == CT - 1),
                                )
                        for j in range(4):
                            ch = half * 4 + j
                            # out = raw * (gamma/Z_c) + x          (exact x)
                            nc.vector.scalar_tensor_tensor(
                                out_t[:, j * CH:(j + 1) * CH],
                                ps_o[j],
                                gz[:, m:m + 1],
                                v_sb[m][:, ch * CH:(ch + 1) * CH],
                                op0=Alu.mult,
                                op1=Alu.add,
                            )
                        nc.sync.dma_start(
                            o_d[b, m * P:(m + 1) * P,
                                half * (N // 2):(half + 1) * (N // 2)],
                            out_t,
                        )

    nc.compile()
    return nc


def _get_program():
    if "nc" not in _CACHE:
        _CACHE["nc"] = _build_program()
    return _CACHE["nc"]


def kernel(x: np.ndarray, gamma: np.ndarray) -> np.ndarray:
    from concourse.bass_utils import run_bass_kernel_spmd

    assert x.shape == (B, C, 64, 64), x.shape
    x = np.ascontiguousarray(x, dtype=np.float32)
    gamma = np.ascontiguousarray(gamma, dtype=np.float32).reshape(1)

    nc = _get_program()
    xs = x.reshape(NCORES, BPC, C, N)
    in_maps = [{"x": xs[i], "gamma": gamma} for i in range(NCORES)]
    res = run_bass_kernel_spmd(nc, in_maps, list(range(NCORES)))
    out = np.empty((NCORES, BPC, C, N), dtype=np.float32)
    for i in range(NCORES):
        out[i] = res.results[i]["out"]
    return out.reshape(B, C, 64, 64)


# revision 5
# speedup vs baseline: 1.2495x; 1.2495x over previous
"""Trainium2 Bass kernel for a CAM (channel-attention) module.

Reference computation (per batch b):
    v    = x[b].reshape(C, H*W)                  # C x N
    e    = v @ v.T                               # C x C Gram matrix
    attn = softmax(rowmax(e) - e, axis=-1)       # == exp(rowmin(e)-e) / rowsum
    out  = gamma * (attn @ v) + x[b]

Sharding: data-parallel over batch B=16 across 8 NeuronCores (2 batches/core,
no cross-core communication).

Per core, per batch:
  - x[b] streamed in fp32 quarter-tiles (residual source, kept exact)
  - DVE casts to bf16 working copies (matmul operands)
  - PE-transpose bf16 128x128 blocks -> resident vT [128n, kt, 512c] (bf16),
    two kt per PSUM bank, single ACT eviction per pair
  - energy: m-outer accumulation (one PSUM bank at a time) so softmax of
    block m overlaps energy of block m+1
  - softmax: rowmin (DVE) + exp with accum_out rowsum (ACT); U unnormalized
  - PE-transpose U -> UT (bf16); raw[c,n] = sum_d U[c,d] v[d,n] (bf16 MMs)
  - eviction fuses normalization+gamma+residual: out = raw*(gamma/Z) + x_fp32
    (so for gamma == 0 the output is bit-exact x)
"""

import numpy as np

P = 128
C = 512
N = 4096
CT = C // P      # 4 c-tiles
NT = N // P      # 32 n-tiles
NP = NT // 2     # 16 transpose pairs
CH = 512         # matmul free-dim chunk
NCH = N // CH    # 8 n-chunks
QN = N // 4      # 1024 quarter width
B = 16
NCORES = 8
BPC = B // NCORES  # batches per core

_CACHE = {}


def _build_program():
    import concourse.bacc as bacc
    import concourse.mybir as mybir
    import concourse.tile as tile
    from concourse.masks import make_identity

    f32 = mybir.dt.float32
    bf16 = mybir.dt.bfloat16
    Alu = mybir.AluOpType
    Act = mybir.ActivationFunctionType

    nc = bacc.Bacc("TRN2", target_bir_lowering=False, debug=False)
    x_d = nc.dram_tensor("x", [BPC, C, N], f32, kind="ExternalInput").ap()
    g_d = nc.dram_tensor("gamma", [1], f32, kind="ExternalInput").ap()
    o_d = nc.dram_tensor("out", [BPC, C, N], f32, kind="ExternalOutput").ap()

    with tile.TileContext(nc) as tc:
        with (
            tc.tile_pool(name="const", bufs=1) as const_pool,
            tc.tile_pool(name="vp", bufs=1) as v_pool,
            tc.tile_pool(name="vhp", bufs=1) as vh_pool,
            tc.tile_pool(name="vtp", bufs=1) as vt_pool,
            tc.tile_pool(name="up", bufs=2) as u_pool,
            tc.tile_pool(name="stat", bufs=2) as st_pool,
            tc.tile_pool(name="outp", bufs=3) as out_pool,
            tc.tile_pool(name="pse", bufs=2, space="PSUM") as ps_e_pool,
            tc.tile_pool(name="pso", bufs=2, space="PSUM") as ps_o_pool,
            tc.tile_pool(name="pstp", bufs=2, space="PSUM") as ps_tp,
        ):
            ident_h = const_pool.tile([P, P], bf16, tag="identh")
            make_identity(nc, ident_h)
            ident_f = const_pool.tile([P, P], f32, tag="identf")
            make_identity(nc, ident_f)
            gamma_bc = const_pool.tile([P, 1], f32, tag="gamma")
            nc.sync.dma_start(gamma_bc, g_d.to_broadcast((P, 1)))

            for b in range(BPC):
                # ---- load x[b] in fp32 quarters, n-interleaved ------------
                v_sb = [[None] * 4 for _ in range(CT)]
                for q in range(4):
                    for ct in range(CT):
                        t = v_pool.tile([P, QN], f32, tag=f"v{ct}q{q}",
                                        bufs=2 if q == 0 else 1,
                                        name=f"v_{b}_{ct}_{q}")
                        nc.sync.dma_start(
                            t, x_d[b, ct * P:(ct + 1) * P, q * QN:(q + 1) * QN])
                        v_sb[ct][q] = t

                # ---- bf16 working copies (DVE casts, by quarter) ----------
                v_h = [[None, None] for _ in range(CT)]
                for q in range(4):
                    for ct in range(CT):
                        h = q // 2
                        if v_h[ct][h] is None:
                            v_h[ct][h] = vh_pool.tile(
                                [P, N // 2], bf16, tag=f"vh{ct}h{h}",
                                bufs=2 if h == 0 else 1,
                                name=f"vh_{b}_{ct}_{h}")
                        nc.vector.tensor_copy(
                            v_h[ct][h][:, (q % 2) * QN:(q % 2 + 1) * QN],
                            v_sb[ct][q])

                # ---- vT: resident bf16, built 2 kt per PSUM bank ----------
                vT = []
                for p in range(NP):
                    ps_t = ps_tp.tile([P, 2, C], bf16, tag="tp",
                                      name=f"ps_tv_{b}_{p}")
                    for sub in range(2):
                        kt = 2 * p + sub
                        h, lk = kt // 16, kt % 16
                        for ct in range(CT):
                            nc.tensor.transpose(
                                ps_t[:, sub, ct * P:(ct + 1) * P],
                                v_h[ct][h][:, lk * P:(lk + 1) * P],
                                ident_h)
                    vt_t = vt_pool.tile([P, 2, C], bf16, tag=f"vt{p}",
                                        name=f"vT_{b}_{p}")
                    nc.scalar.activation(vt_t, ps_t, Act.Copy)
                    vT.append(vt_t)

                mins = st_pool.tile([P, CT], f32, tag="mins", name=f"mins_{b}")
                zsum = st_pool.tile([P, CT], f32, tag="zsum", name=f"zsum_{b}")
                gz = st_pool.tile([P, CT], f32, tag="gz", name=f"gz_{b}")
                u_sb = u_pool.tile([P, CT, C], f32, tag="u", name=f"u_{b}")
                ut_sb = u_pool.tile([P, CT, C], bf16, tag="ut", name=f"ut_{b}")

                # ---- energy, m-outer: softmax(m) overlaps energy(m+1) -----
                for m in range(CT):
                    ps = ps_e_pool.tile([P, CH], f32, tag="e",
                                        name=f"ps_e_{b}_{m}")
                    for p in range(NP):
                        for sub in range(2):
                            nc.tensor.matmul(
                                ps,
                                vT[p][:, sub, m * P:(m + 1) * P],
                                vT[p][:, sub, :],
                                start=(p == 0 and sub == 0),
                                stop=(p == NP - 1 and sub == 1))
                    nc.vector.tensor_reduce(
                        mins[:, m:m + 1], ps, axis=mybir.AxisListType.X,
                        op=Alu.min)
                    nc.scalar.activation(
                        u_sb[:, m, :], ps, Act.Exp,
                        bias=mins[:, m:m + 1], scale=-1.0,
                        accum_out=zsum[:, m:m + 1])
                    nc.vector.reciprocal(gz[:, m:m + 1], zsum[:, m:m + 1])
                    nc.vector.tensor_tensor(
                        gz[:, m:m + 1], gz[:, m:m + 1], gamma_bc, Alu.mult)

                # ---- UT = U.T (16 PE transposes, fp32 -> bf16) ------------
                for kt in range(CT):
                    ps_u = ps_tp.tile([P, CH], f32, tag="tp",
                                      name=f"ps_ut_{b}_{kt}")
                    for m in range(CT):
                        nc.tensor.transpose(
                            ps_u[:, m * P:(m + 1) * P],
                            u_sb[:, m, kt * P:(kt + 1) * P],
                            ident_f)
                    nc.scalar.activation(ut_sb[:, kt, :], ps_u, Act.Copy)

                # ---- raw[c,n] = sum_d U[c,d] v[d,n]; fused eviction -------
                for m in range(CT):
                    for pair in range(4):          # pairs of 512-chunks
                        out_t = out_pool.tile([P, QN], f32, tag="o",
                                              name=f"o_{b}_{m}_{pair}")
                        for sub in range(2):
                            ch = pair * 2 + sub
                            ps_o = ps_o_pool.tile([P, CH], f32, tag="o",
                                                  name=f"ps_o_{b}_{m}_{ch}")
                            for kt in range(CT):
                                nc.tensor.matmul(
                                    ps_o,
                                    ut_sb[:, kt, m * P:(m + 1) * P],
                                    v_h[kt][ch // 4][:, (ch % 4) * CH:
                                                     (ch % 4 + 1) * CH],
                                    start=(kt == 0),
                                    stop=(kt == CT - 1))
                            # out = raw * (gamma/Z_c) + x       (exact x)
                            nc.vector.scalar_tensor_tensor(
                                out_t[:, sub * CH:(sub + 1) * CH],
                                ps_o,
                                gz[:, m:m + 1],
                                v_sb[m][ch // 2][:, (ch % 2) * CH:
                                                 (ch % 2 + 1) * CH],
                                op0=Alu.mult,
                                op1=Alu.add)
                        nc.sync.dma_start(
                            o_d[b, m * P:(m + 1) * P,
                                pair * QN:(pair + 1) * QN],
                            out_t)

    nc.compile()
    return nc


def _get_program():
    if "nc" not in _CACHE:
        _CACHE["nc"] = _build_program()
    return _CACHE["nc"]


def kernel(x: np.ndarray, gamma: np.ndarray) -> np.ndarray:
    from concourse.bass_utils import run_bass_kernel_spmd

    assert x.shape == (B, C, 64, 64), x.shape
    x = np.ascontiguousarray(x, dtype=np.float32)
    gamma = np.ascontiguousarray(gamma, dtype=np.float32).reshape(1)

    nc = _get_program()
    xs = x.reshape(NCORES, BPC, C, N)
    in_maps = [{"x": xs[i], "gamma": gamma} for i in range(NCORES)]
    res = run_bass_kernel_spmd(nc, in_maps, list(range(NCORES)))
    out = np.empty((NCORES, BPC, C, N), dtype=np.float32)
    for i in range(NCORES):
        out[i] = res.results[i]["out"]
    return out.reshape(B, C, 64, 64)


# revision 7
# speedup vs baseline: 1.4374x; 1.1504x over previous
"""Trainium2 Bass kernel for a CAM (channel-attention) module.

Reference computation (per batch b):
    v    = x[b].reshape(C, H*W)                  # C x N
    e    = v @ v.T                               # C x C Gram matrix
    attn = softmax(rowmax(e) - e, axis=-1)       # == exp(rowmin(e)-e) / rowsum
    out  = gamma * (attn @ v) + x[b]

Sharding: data-parallel over batch B=16 across 8 NeuronCores (2 batches/core,
no cross-core communication).

Per core, per batch:
  - x[b] streamed in fp32 quarter-tiles (residual source, kept exact)
  - DVE casts to bf16 working copies (matmul operands)
  - PE-transpose bf16 128x128 blocks -> resident vT [128n, kt, 512c] (bf16),
    two kt per PSUM bank, single ACT eviction per pair
  - energy: m-outer accumulation (one PSUM bank at a time) so softmax of
    block m overlaps energy of block m+1
  - softmax: rowmin (DVE) + exp with accum_out rowsum (ACT); U unnormalized
  - PE-transpose U -> UT (bf16); raw[c,n] = sum_d U[c,d] v[d,n] (bf16 MMs)
  - eviction fuses normalization+gamma+residual: out = raw*(gamma/Z) + x_fp32
    (so for gamma == 0 the output is bit-exact x)
"""

import numpy as np

P = 128
C = 512
N = 4096
CT = C // P      # 4 c-tiles
NT = N // P      # 32 n-tiles
NP = NT // 2     # 16 transpose pairs
CH = 512         # matmul free-dim chunk
NCH = N // CH    # 8 n-chunks
QN = N // 4      # 1024 quarter width
B = 16
NCORES = 8
BPC = B // NCORES  # batches per core

_CACHE = {}


def _build_program():
    import concourse.bacc as bacc
    import concourse.mybir as mybir
    import concourse.tile as tile
    from concourse.masks import make_identity

    f32 = mybir.dt.float32
    bf16 = mybir.dt.bfloat16
    Alu = mybir.AluOpType
    Act = mybir.ActivationFunctionType

    nc = bacc.Bacc("TRN2", target_bir_lowering=False, debug=False)
    x_d = nc.dram_tensor("x", [BPC, C, N], f32, kind="ExternalInput").ap()
    g_d = nc.dram_tensor("gamma", [1], f32, kind="ExternalInput").ap()
    o_d = nc.dram_tensor("out", [BPC, C, N], f32, kind="ExternalOutput").ap()

    with tile.TileContext(nc) as tc:
        with (
            tc.tile_pool(name="const", bufs=1) as const_pool,
            tc.tile_pool(name="vp", bufs=1) as v_pool,
            tc.tile_pool(name="vhp", bufs=1) as vh_pool,
            tc.tile_pool(name="vtp", bufs=1) as vt_pool,
            tc.tile_pool(name="up", bufs=2) as u_pool,
            tc.tile_pool(name="stat", bufs=2) as st_pool,
            tc.tile_pool(name="outp", bufs=3) as out_pool,
            tc.tile_pool(name="pse", bufs=2, space="PSUM") as ps_e_pool,
            tc.tile_pool(name="pso", bufs=4, space="PSUM") as ps_o_pool,
            tc.tile_pool(name="pstp", bufs=2, space="PSUM") as ps_tp,
        ):
            ident_h = const_pool.tile([P, P], bf16, tag="identh")
            make_identity(nc, ident_h)
            ident_f = const_pool.tile([P, P], f32, tag="identf")
            make_identity(nc, ident_f)
            gamma_bc = const_pool.tile([P, 1], f32, tag="gamma")
            nc.sync.dma_start(gamma_bc, g_d.to_broadcast((P, 1)))

            for b in range(BPC):
                # ---- load x[b] in fp32 quarters, n-interleaved ------------
                v_sb = [[None] * 4 for _ in range(CT)]
                for q in range(4):
                    for ct in range(CT):
                        t = v_pool.tile([P, QN], f32, tag=f"v{ct}q{q}",
                                        bufs=2 if q == 0 else 1,
                                        name=f"v_{b}_{ct}_{q}")
                        nc.sync.dma_start(
                            t, x_d[b, ct * P:(ct + 1) * P, q * QN:(q + 1) * QN])
                        v_sb[ct][q] = t

                # ---- bf16 working copies (DVE casts, by quarter) ----------
                v_h = [[None, None] for _ in range(CT)]
                for q in range(4):
                    for ct in range(CT):
                        h = q // 2
                        if v_h[ct][h] is None:
                            v_h[ct][h] = vh_pool.tile(
                                [P, N // 2], bf16, tag=f"vh{ct}h{h}",
                                bufs=2 if h == 0 else 1,
                                name=f"vh_{b}_{ct}_{h}")
                        nc.vector.tensor_copy(
                            v_h[ct][h][:, (q % 2) * QN:(q % 2 + 1) * QN],
                            v_sb[ct][q])

                # ---- vT: resident bf16, built 2 kt per PSUM bank ----------
                vT = []
                for p in range(NP):
                    ps_t = ps_tp.tile([P, 2, C], bf16, tag="tp",
                                      name=f"ps_tv_{b}_{p}")
                    for sub in range(2):
                        kt = 2 * p + sub
                        h, lk = kt // 16, kt % 16
                        for ct in range(CT):
                            nc.tensor.transpose(
                                ps_t[:, sub, ct * P:(ct + 1) * P],
                                v_h[ct][h][:, lk * P:(lk + 1) * P],
                                ident_h)
                    vt_t = vt_pool.tile([P, 2, C], bf16, tag=f"vt{p}",
                                        name=f"vT_{b}_{p}")
                    nc.scalar.activation(vt_t, ps_t, Act.Copy)
                    vT.append(vt_t)

                mins = st_pool.tile([P, CT], f32, tag="mins", name=f"mins_{b}")
                zsum = st_pool.tile([P, CT], f32, tag="zsum", name=f"zsum_{b}")
                gz = st_pool.tile([P, CT], f32, tag="gz", name=f"gz_{b}")
                e_sb = u_pool.tile([P, CT, C], f32, tag="es", bufs=1,
                                   name=f"e_{b}")
                u_sb = u_pool.tile([P, CT, C], f32, tag="u", bufs=1,
                                   name=f"u_{b}")
                ut_sb = u_pool.tile([P, CT, C], bf16, tag="ut", name=f"ut_{b}")

                # ---- energy, m-outer, upper-triangular blocks only --------
                # e is symmetric: compute e[m-rows, d >= m*128]; reconstruct
                # the lower blocks by PE-transposing e[j-rows, m-cols], j<m.
                for m in range(CT):
                    W = C - m * P
                    ps = ps_e_pool.tile([P, CH], f32, tag="e",
                                        name=f"ps_e_{b}_{m}")
                    for p in range(NP):
                        for sub in range(2):
                            nc.tensor.matmul(
                                ps[:, :W],
                                vT[p][:, sub, m * P:(m + 1) * P],
                                vT[p][:, sub, m * P:],
                                start=(p == 0 and sub == 0),
                                stop=(p == NP - 1 and sub == 1))
                    nc.scalar.activation(e_sb[:, m, m * P:], ps[:, :W],
                                         Act.Copy)
                    if m > 0:
                        ps_x = ps_tp.tile([P, CH], f32, tag="tp",
                                          name=f"ps_ex_{b}_{m}")
                        for j in range(m):
                            nc.tensor.transpose(
                                ps_x[:, j * P:(j + 1) * P],
                                e_sb[:, j, m * P:(m + 1) * P],
                                ident_f)
                        nc.scalar.activation(e_sb[:, m, :m * P],
                                             ps_x[:, :m * P], Act.Copy)
                    nc.vector.tensor_reduce(
                        mins[:, m:m + 1], e_sb[:, m, :],
                        axis=mybir.AxisListType.X, op=Alu.min)
                    nc.scalar.activation(
                        u_sb[:, m, :], e_sb[:, m, :], Act.Exp,
                        bias=mins[:, m:m + 1], scale=-1.0,
                        accum_out=zsum[:, m:m + 1])
                    nc.vector.reciprocal(gz[:, m:m + 1], zsum[:, m:m + 1])
                    nc.vector.tensor_tensor(
                        gz[:, m:m + 1], gz[:, m:m + 1], gamma_bc, Alu.mult)

                # ---- UT = U.T (16 PE transposes, fp32 -> bf16) ------------
                for kt in range(CT):
                    ps_u = ps_tp.tile([P, CH], f32, tag="tp",
                                      name=f"ps_ut_{b}_{kt}")
                    for m in range(CT):
                        nc.tensor.transpose(
                            ps_u[:, m * P:(m + 1) * P],
                            u_sb[:, m, kt * P:(kt + 1) * P],
                            ident_f)
                    nc.scalar.activation(ut_sb[:, kt, :], ps_u, Act.Copy)

                # ---- raw[c,n] = sum_d U[c,d] v[d,n]; fused eviction -------
                for m in range(CT):
                    for pair in range(4):          # pairs of 512-chunks
                        out_t = out_pool.tile([P, QN], f32, tag="o",
                                              name=f"o_{b}_{m}_{pair}")
                        for sub in range(2):
                            ch = pair * 2 + sub
                            ps_o = ps_o_pool.tile([P, CH], f32, tag="o",
                                                  name=f"ps_o_{b}_{m}_{ch}")
                            for kt in range(CT):
                                nc.tensor.matmul(
                                    ps_o,
                                    ut_sb[:, kt, m * P:(m + 1) * P],
                                    v_h[kt][ch // 4][:, (ch % 4) * CH:
                                                     (ch % 4 + 1) * CH],
                                    start=(kt == 0),
                                    stop=(kt == CT - 1))
                            # out = raw * (gamma/Z_c) + x       (exact x)
                            nc.vector.scalar_tensor_tensor(
                                out_t[:, sub * CH:(sub + 1) * CH],
                                ps_o,
                                gz[:, m:m + 1],
                                v_sb[m][ch // 2][:, (ch % 2) * CH:
                                                 (ch % 2 + 1) * CH],
                                op0=Alu.mult,
                                op1=Alu.add)
                        nc.sync.dma_start(
                            o_d[b, m * P:(m + 1) * P,
                                pair * QN:(pair + 1) * QN],
                            out_t)

    nc.compile()
    return nc


def _get_program():
    if "nc" not in _CACHE:
        _CACHE["nc"] = _build_program()
    return _CACHE["nc"]


def kernel(x: np.ndarray, gamma: np.ndarray) -> np.ndarray:
    from concourse.bass_utils import run_bass_kernel_spmd

    assert x.shape == (B, C, 64, 64), x.shape
    x = np.ascontiguousarray(x, dtype=np.float32)
    gamma = np.ascontiguousarray(gamma, dtype=np.float32).reshape(1)

    nc = _get_program()
    xs = x.reshape(NCORES, BPC, C, N)
    in_maps = [{"x": xs[i], "gamma": gamma} for i in range(NCORES)]
    res = run_bass_kernel_spmd(nc, in_maps, list(range(NCORES)))
    out = np.empty((NCORES, BPC, C, N), dtype=np.float32)
    for i in range(NCORES):
        out[i] = res.results[i]["out"]
    return out.reshape(B, C, 64, 64)
